# revision 44
# baseline (speedup 1.0000x reference)
"""Dynamic depthwise-conv branch (DynamicConvBranch) Trainium2 kernel.

Problem (hardcoded shapes):
  x  [16, 32, 384, 384] f32
  w1 [32, 128], b1 [128], w2 [128, 288], b2 [288]
  out[b,c] = conv2d_same3x3(x[b,c], k[b,c]) where
  k = reshape(relu(mean_hw(x) @ w1 + b1) @ w2 + b2, [B, 32, 3, 3])

Strategy: pure data parallel over batch (2 samples per core, 8 cores).
Per sample, x is held resident in SBUF as 4 row-strips x 32 channels of
[98, 386] tiles (1-row halos, zero-padded W edges).  Channel means are
computed with TensorE (ones-vector matmuls accumulated in PSUM), the
kernel-generator MLP runs as two small matmuls, and the depthwise 3x3
conv is computed as 3 PSUM-accumulated matmuls per (channel, strip)
using per-channel tridiagonal band matrices (built on VectorE from
host-baked 0/1 diagonal masks scaled by the generated kernel values).
"""

import numpy as np

B, C, H, W = 16, 32, 384, 384
NK = 32
HID = 128
KK = 3
N_CORES = 8
B_PER_CORE = B // N_CORES

GC = 8           # channels per DMA group
NG = C // GC     # 4 groups
SH = 96          # output rows per strip
NS = H // SH     # 4 strips
KP = SH + 2      # input rows per strip tile (with halo) = 98
WP = W + 2       # padded width: cols 0 and 385 are zero

_CACHE = {}


def _build_nc():
    from contextlib import ExitStack
    from concourse import bass, bacc, tile
    from concourse.bass import mybir

    f32 = mybir.dt.float32
    f32r = mybir.dt.float32r
    Alu = mybir.AluOpType
    Act = mybir.ActivationFunctionType

    nc = bacc.Bacc()

    x_d = nc.dram_tensor("x", [B_PER_CORE, C, H, W], f32r, kind="ExternalInput")
    w1_d = nc.dram_tensor("w1", [C, HID], f32, kind="ExternalInput")
    b1_d = nc.dram_tensor("b1", [HID], f32, kind="ExternalInput")
    w2_d = nc.dram_tensor("w2", [HID, NK * KK * KK], f32, kind="ExternalInput")
    b2_d = nc.dram_tensor("b2", [NK * KK * KK], f32, kind="ExternalInput")
    out_d = nc.dram_tensor("out", [B_PER_CORE, NK, H, W], f32, kind="ExternalOutput")

    # Host-baked diagonal masks: masks[dh][p, m] = 1 iff p == m + dh.
    # A band matrix A[p, m] = k[dh = p - m] is then
    #   A = k0*masks[0] + k1*masks[1] + k2*masks[2].
    import ml_dtypes
    masks_np = np.zeros((KP, KK, SH), dtype=np.float32)
    for dh in range(KK):
        for m in range(SH):
            masks_np[m + dh, dh, m] = 1.0
    masks_d = nc.inline_tensor(masks_np.astype(ml_dtypes.bfloat16), name="bandmasks")
    onesp_np = np.ones((KP, 1), dtype=np.float32)
    onesp_np[0, 0] = 0.0
    onesp_np[KP - 1, 0] = 0.0
    onesp_d = nc.inline_tensor(onesp_np, name="onesp")
    zrow_d = nc.inline_tensor(np.zeros((1, GC * WP), dtype=np.float32), name="zrow")
    zcol_d = nc.inline_tensor(np.zeros((KP, GC, 2), dtype=np.float32), name="zcol")

    with tile.TileContext(nc) as tc, ExitStack() as ctx:
        xpool_s0 = ctx.enter_context(tc.tile_pool(name="xs0", bufs=NG))
        xpool_mid = ctx.enter_context(tc.tile_pool(name="xmid", bufs=NG * 2))
        xpool_s3 = ctx.enter_context(tc.tile_pool(name="xs3", bufs=NG))
        cpool = ctx.enter_context(tc.tile_pool(name="const", bufs=1))
        mpool = ctx.enter_context(tc.tile_pool(name="mlp", bufs=1))
        kvpool = ctx.enter_context(tc.tile_pool(name="kv", bufs=1))
        kbpool = ctx.enter_context(tc.tile_pool(name="kb", bufs=1))
        apool = ctx.enter_context(tc.tile_pool(name="amat", bufs=5))
        cspool = ctx.enter_context(tc.tile_pool(name="csum", bufs=NS * NG + 2))
        opool = ctx.enter_context(tc.tile_pool(name="ostage", bufs=2))
        pp_ps = ctx.enter_context(
            tc.tile_pool(name="poolps", bufs=2, space=bass.MemorySpace.PSUM))
        cv_ps = ctx.enter_context(
            tc.tile_pool(name="convps", bufs=3, space=bass.MemorySpace.PSUM))

        # --- one-time constants ---
        bf16 = mybir.dt.bfloat16
        masks = cpool.tile([KP, KK, SH], bf16)
        nc.sync.dma_start(masks[:], masks_d[:])

        onesp = cpool.tile([KP, 1], f32r)         # interior mask for pooling
        nc.sync.dma_start(onesp[:], onesp_d[:].bitcast(f32r))

        ones1 = cpool.tile([1, HID], f32)         # for partition broadcast
        nc.vector.memset(ones1[:], 1.0)

        w1b = cpool.tile([C + 1, HID], f32)       # [w1; b1]
        nc.sync.dma_start(w1b[0:C, :], w1_d[:])
        nc.sync.dma_start(w1b[C:C + 1, :], b1_d[:].unsqueeze(0))

        w2s = cpool.tile([HID, NK * KK * KK], f32)
        nc.sync.dma_start(w2s[:], w2_d[:])
        b2s = cpool.tile([1, NK * KK * KK], f32)
        nc.sync.dma_start(b2s[:], b2_d[:].unsqueeze(0))

        # tiny warm-up matmul: absorbs the PE preamble wait + const DMA lane
        # ticks so real matmuls carry few semaphore waits (ISA slot limit).
        warm_ps = pp_ps.tile([1, 1], f32, tag="pool")
        nc.tensor.matmul(warm_ps[:], onesp[:].bitcast(f32), onesp[:].bitcast(f32),
                         start=True, stop=True)

        for b in range(B_PER_CORE):
            # --- load x resident (8 channels per DMA) + pooled sums on PE ---
            prow_ps = pp_ps.tile([1, C], f32, tag="pool")
            xt = {}
            csum = {}
            for g in range(NG):
                c0 = g * GC
                for s in range(NS):
                    r0 = s * SH
                    xsrc = x_d[b, c0:c0 + GC]
                    if s == 0:
                        t = xpool_s0.tile([KP, GC, WP], f32r, tag="xs0")
                        if b == 0:
                            nc.sync.dma_start(
                                t[0:1, :, :], zrow_d[:].bitcast(f32r).rearrange(
                                    "p (c w) -> p c w", c=GC))  # row -1 = pad
                        nc.sync.dma_start(
                            t[1:KP, :, 1:W + 1],
                            xsrc[:, 0:KP - 1, :].rearrange("c r w -> r c w"))
                    elif s == NS - 1:
                        t = xpool_s3.tile([KP, GC, WP], f32r, tag="xs3")
                        if b == 0:
                            nc.sync.dma_start(
                                t[KP - 1:KP, :, :], zrow_d[:].bitcast(f32r).rearrange(
                                    "p (c w) -> p c w", c=GC))  # row H = pad
                        nc.sync.dma_start(
                            t[0:KP - 1, :, 1:W + 1],
                            xsrc[:, r0 - 1:H, :].rearrange("c r w -> r c w"))
                    else:
                        t = xpool_mid.tile([KP, GC, WP], f32r, tag="xmid")
                        nc.sync.dma_start(
                            t[:, :, 1:W + 1],
                            xsrc[:, r0 - 1:r0 + KP - 1, :].rearrange("c r w -> r c w"))
                    if b == 0:
                        # zero the W-pad columns (0 and 385); loads never
                        # touch them, so slot reuse keeps them zero
                        nc.sync.dma_start(t[:, :, 0:1],
                                          zcol_d[:, :, 0:1].bitcast(f32r))
                        nc.sync.dma_start(t[:, :, WP - 1:WP],
                                          zcol_d[:, :, 1:2].bitcast(f32r))
                    cs = cspool.tile([KP, GC], f32, tag="cs")
                    nc.vector.tensor_reduce(cs[:], t[:, :, :],
                                            mybir.AxisListType.X, Alu.add)
                    csum[(g, s)] = cs
                    xt[(g, s)] = t
                for s in range(NS):
                    nc.tensor.matmul(prow_ps[0:1, c0:c0 + GC], onesp[:].bitcast(f32),
                                     csum[(g, s)][:],
                                     start=(s == 0), stop=(s == NS - 1))

            # --- kernel-generator MLP ---
            pmrow = mpool.tile([1, C], f32)
            nc.scalar.activation(pmrow[:], prow_ps[:], Act.Copy)
            pmt_ps = pp_ps.tile([C, 1], f32, tag="pool")
            nc.tensor.matmul(pmt_ps[:], pmrow[:], ones1[0:1, 0:1],
                             start=True, stop=True)
            pm = mpool.tile([C + 1, 1], f32)
            nc.scalar.activation(pm[0:C, :], pmt_ps[:], Act.Copy,
                                 scale=1.0 / (H * W))
            nc.vector.memset(pm[C:C + 1, :], 1.0)

            h1_ps = pp_ps.tile([HID, 1], f32, tag="pool")
            nc.tensor.matmul(h1_ps[:], w1b[:], pm[:], start=True, stop=True)
            h1s = mpool.tile([HID, 1], f32)
            nc.scalar.activation(h1s[:], h1_ps[:], Act.Relu)

            k_ps = pp_ps.tile([1, NK * KK * KK], f32, tag="pool")
            nc.tensor.matmul(k_ps[:], h1s[:], w2s[:], start=True, stop=True)
            kvec = kvpool.tile([1, NK * KK * KK], f32)
            nc.vector.tensor_tensor(kvec[:], k_ps[:], b2s[:], Alu.add)

            kb_ps = pp_ps.tile([HID, NK * KK * KK], f32, tag="pool")
            nc.tensor.matmul(kb_ps[:], ones1[:], kvec[:], start=True, stop=True)
            kb = kbpool.tile([HID, NK * KK * KK], f32)
            nc.scalar.activation(kb[:], kb_ps[:], Act.Copy)

            # --- depthwise conv: band-matrix matmuls per (channel, strip) ---
            for c in range(C):
                g, cc = divmod(c, GC)
                amat = []
                for dw in range(KK):
                    a = apool.tile([KP, SH], f32r, tag="amat")
                    amat.append(a)
                    ks = lambda dh: kb[0:KP, c * 9 + dh * 3 + dw:c * 9 + dh * 3 + dw + 1]
                    nc.vector.tensor_scalar(a[:], masks[:, 0, :], ks(0), None,
                                            op0=Alu.mult)
                    nc.vector.scalar_tensor_tensor(a[:], masks[:, 1, :], ks(1),
                                                   a[:], op0=Alu.mult, op1=Alu.add)
                    nc.vector.scalar_tensor_tensor(a[:], masks[:, 2, :], ks(2),
                                                   a[:], op0=Alu.mult, op1=Alu.add)
                for j in range(NS // 2):
                    ob = opool.tile([SH, 2, W], f32, tag="ob")
                    o_ps = cv_ps.tile([SH, 2, 512], f32, tag="cv")  # 2 banks
                    for half in range(2):
                        s = 2 * j + half
                        t = xt[(g, s)]
                        for dw in range(KK):
                            nc.tensor.matmul(o_ps[:, half, 0:W], amat[dw][:],
                                             t[:, cc, dw:dw + W],
                                             start=(dw == 0), stop=(dw == KK - 1))
                    nc.scalar.activation(ob[:], o_ps[:, :, 0:W], Act.Copy)
                    nc.sync.dma_start(
                        out_d[b, c, 2 * j * SH:(2 * j + 2) * SH, :].rearrange(
                            "(s p) w -> p s w", s=2),
                        ob[:])

    nc.compile()
    return nc


def _run(inputs, trace=False):
    from concourse.bass_utils import run_bass_kernel_spmd

    if "nc" not in _CACHE:
        _CACHE["nc"] = _build_nc()
    nc = _CACHE["nc"]

    x = np.ascontiguousarray(inputs["x"], dtype=np.float32)
    w1 = np.ascontiguousarray(inputs["w1"], dtype=np.float32)
    b1 = np.ascontiguousarray(inputs["b1"], dtype=np.float32)
    w2 = np.ascontiguousarray(inputs["w2"], dtype=np.float32)
    b2 = np.ascontiguousarray(inputs["b2"], dtype=np.float32)

    in_maps = []
    for i in range(N_CORES):
        in_maps.append({
            "x": x[i * B_PER_CORE:(i + 1) * B_PER_CORE],
            "w1": w1, "b1": b1, "w2": w2, "b2": b2,
        })
    res = run_bass_kernel_spmd(nc, in_maps, list(range(N_CORES)), trace=trace)
    out = np.concatenate([res.results[i]["out"] for i in range(N_CORES)], axis=0)
    return out, res


def kernel(**inputs):
    out, _ = _run(inputs, trace=False)
    return out
